# revision 6
# baseline (speedup 1.0000x reference)
"""Contrastive queue loss kernel for 8 Trainium2 NeuronCores.

Reference computation (all fp32):
    pos[j,b]    = V[j,b,:] . L[b,:] / T                  (J=2, B=256, F=128)
    qlog[j,b,q] = V[j,b,:] . queue[q,:] / T              (Q=65536)
    denom[j,b]  = log( sum_i exp(pos[j,i]) + sum_q exp(qlog[j,b,q]) )
    loss        = -sum_{j,b} (pos[j,b] - denom[j,b]) / B

Sharding: queue split along Q across 8 cores (8192 rows each); V/L replicated.
Each core emits pos[jb] (fp32 dot products) and its partial
sum_q exp(10*logit) per jb. Host combines partials in float64.

Key structural choice: sum_q exp(...) is invariant to any permutation of q,
so the queue shard is DMAed CONTIGUOUSLY (partition p holds rows
p*s_range+s of the chunk -> 2-4KB contiguous per partition line, near-peak
HBM bandwidth) instead of a q-interleaved gather; the PE transposes then
yield a q-permuted [f, q] operand, which is fine.

Per-core dataflow:
  DMA queue chunk (contiguous fp32) -> DVE cast to bf16
  -> PE 128x128 identity-matmul transposes into PSUM -> DVE copy to SBUF
  -> PE matmul against persistent V2T (bf16) -> logits in PSUM (fp32)
  -> ACT exp(10x) in place with fused accumulation (accum_out)
  -> DVE reduces partial columns, DMA out [2,128,4].
Scalar-engine (exp) time is the roofline: 16 ACT ops, per-tile group
schedule [1,3,6,6] slices of 512 so ACT starts as soon as the first
512 queue rows land. partition-id plumbing disabled (kernel never reads
it) to trim the per-engine TENSOR_LOAD preamble.
"""

import numpy as np

J, B, F, Q = 2, 256, 128, 65536
NCORES = 8
QC = Q // NCORES          # 8192 queue rows per core
JB = J * B                # 512
INV_T = 10.0
NT = JB // 128            # 4 jb tiles of 128
NSLICE = QC // 512        # 16 rhs slices of 512 q rows

# (row_start, nrows) queue chunks, contiguous per partition.
CHUNKS = [(0, 512), (512, 512)] + [(r, 1024) for r in range(1024, QC, 1024)]
# Per-jb-tile ACT group schedule (slices of 512 q rows; sums to 16).
# Small first group so ACT starts right after the first chunk lands.
# Matmul outputs must be fp32, so ops cap at 1536 (3 PSUM banks x2 bufs
# + 2 transpose-staging banks = 8).
GROUPS = [(0, 1), (1, 3), (4, 3), (7, 3), (10, 3), (13, 3)]
NG = len(GROUPS)
N_WARMUP = 8              # dummy PE matmuls to lift the HAM clock gate

_STATE = {}


def _build():
    import concourse.tile as tile
    from concourse import bacc, masks, mybir

    f32 = mybir.dt.float32
    bf16 = mybir.dt.bfloat16
    nc = bacc.Bacc("TRN2", target_bir_lowering=False, debug=False,
                   num_devices=None, enable_partition_id=False)

    v2_d = nc.dram_tensor("V2", (JB, F), f32, kind="ExternalInput")
    l_d = nc.dram_tensor("L", (B, F), f32, kind="ExternalInput")
    q_d = nc.dram_tensor("queue", (QC, F), f32, kind="ExternalInput")
    # out[0, p, t] = pos_raw[jb = t*128 + p]
    # out[1, p, t] = sum over this core's queue shard of exp(10 * logit[jb, q])
    out_d = nc.dram_tensor("out", (2, 128, NT), f32, kind="ExternalOutput")

    with tile.TileContext(nc) as tc:
        with (
            tc.tile_pool(name="const", bufs=1) as const_pool,
            tc.tile_pool(name="vl", bufs=1) as vl_pool,
            tc.tile_pool(name="qt", bufs=3) as qt_pool,
            tc.tile_pool(name="qtb", bufs=3) as qtb_pool,
            tc.tile_pool(name="qts", bufs=6) as qts_pool,
            tc.tile_pool(name="trash", bufs=2) as trash_pool,
            tc.tile_pool(name="res", bufs=1) as res_pool,
            tc.tile_pool(name="pslog", bufs=2, space="PSUM") as pslog_pool,
            tc.tile_pool(name="pst", bufs=2, space="PSUM") as pst_pool,
        ):
            # PE clock warmup: dummy matmuls keep the PE array clocked
            # while the first input DMAs are in flight.
            wsrc = const_pool.tile([128, 128], bf16, tag="wsrc")
            nc.vector.memset(wsrc[:], 0.0)
            lgw = pslog_pool.tile([128, 128], f32, tag="pslog")
            for _ in range(N_WARMUP):
                nc.tensor.matmul(lgw[:], lhsT=wsrc[:], rhs=wsrc[:],
                                 start=True, stop=True)

            identb = const_pool.tile([128, 128], bf16, tag="identb")
            masks.make_identity(nc, identb[:])

            # V2T [f=128, jb=512] bf16 (needed by the first matmul group)
            vt_all = vl_pool.tile([128, JB], f32)      # [p, (t f)] natural V2
            nc.sync.dma_start(
                vt_all[:].rearrange("p (t f) -> p t f", f=F),
                v2_d.ap().rearrange("(t p) f -> p t f", p=128))
            vtb = vl_pool.tile([128, JB], bf16)
            nc.vector.tensor_copy(vtb[:], vt_all[:])
            pv = pst_pool.tile([128, 1024], bf16, tag="pst")
            for t in range(NT):
                nc.tensor.transpose(
                    pv[:, t * 128:(t + 1) * 128],
                    vtb[:, t * 128:(t + 1) * 128], identb[:])
            v2tb = vl_pool.tile([128, JB], bf16)       # [f, jb]
            nc.vector.tensor_copy(v2tb[:], pv[:, :JB])

            # ---- stream queue chunks: load (contiguous), cast, transpose ----
            slices = []                                # 16 x [f=128, q=512] bf16
            for r0, nr in CHUNKS:
                sr = nr // 128                         # rows per partition
                qt = qt_pool.tile([128, nr], f32, tag="qt")
                nc.sync.dma_start(
                    qt[:].rearrange("p (s f) -> p s f", f=F),
                    q_d.ap()[r0:r0 + nr, :].rearrange(
                        "(p s) f -> p s f", s=sr))
                qtb = qtb_pool.tile([128, nr], bf16, tag="qtb")
                nc.vector.tensor_copy(qtb[:], qt[:])
                pt = pst_pool.tile([128, nr], bf16, tag="pst")
                for s in range(nr // 128):
                    nc.tensor.transpose(
                        pt[:, s * 128:(s + 1) * 128],
                        qtb[:, s * 128:(s + 1) * 128], identb[:])
                qts = qts_pool.tile([128, nr], bf16, tag="qts")
                nc.vector.tensor_copy(qts[:], pt[:])
                for s in range(nr // 512):
                    slices.append(qts[:, s * 512:(s + 1) * 512])

            # ---- logits (bf16 in PSUM) + fused exp/accumulate ----
            # acc[p, t*NG + g] = partial sum for jb tile t, ACT group g
            acc = res_pool.tile([128, NT * NG], f32)
            for gi, (s0, ns) in enumerate(GROUPS):
                for t in range(NT):
                    lg = pslog_pool.tile([128, 512 * ns], f32, tag="pslog")
                    for k in range(ns):
                        nc.tensor.matmul(
                            lg[:, k * 512:(k + 1) * 512],
                            lhsT=v2tb[:, t * 128:(t + 1) * 128],
                            rhs=slices[s0 + k], start=True, stop=True)
                    col = t * NG + gi
                    nc.scalar.activation(
                        lg[:], lg[:], mybir.ActivationFunctionType.Exp,
                        scale=INV_T, accum_out=acc[:, col:col + 1])

            # ---- pos[jb] (fp32, off the critical path) ----
            lt = vl_pool.tile([128, B], f32)           # [p, (u f)] natural L
            nc.sync.dma_start(
                lt[:].rearrange("p (u f) -> p u f", f=F),
                l_d.ap().rearrange("(u p) f -> p u f", p=128))
            pos_sb = res_pool.tile([128, NT], f32)
            junk = trash_pool.tile([128, 128], f32, tag="junk")
            for t in range(NT):
                u = t % (B // 128)
                nc.vector.tensor_mul(
                    junk[:],
                    vt_all[:, t * 128:(t + 1) * 128],
                    lt[:, u * 128:(u + 1) * 128])
                nc.vector.tensor_reduce(
                    out=pos_sb[:, t:t + 1], in_=junk[:],
                    axis=mybir.AxisListType.X, op=mybir.AluOpType.add)

            # ---- finalize: reduce partials over groups, DMA out ----
            s_sb = res_pool.tile([128, NT], f32)
            for t in range(NT):
                nc.vector.tensor_reduce(
                    out=s_sb[:, t:t + 1],
                    in_=acc[:, t * NG:(t + 1) * NG],
                    axis=mybir.AxisListType.X, op=mybir.AluOpType.add)
            nc.sync.dma_start(out_d.ap()[0], pos_sb[:])
            nc.sync.dma_start(out_d.ap()[1], s_sb[:])

    nc.compile()
    return nc


def _run(in_maps, trace=False, **kwargs):
    from concourse.bass_utils import run_bass_kernel_spmd
    if "nc" not in _STATE:
        _STATE["nc"] = _build()
    return run_bass_kernel_spmd(_STATE["nc"], in_maps, list(range(NCORES)),
                                trace=trace, **kwargs)


def _make_in_maps(V, L, queue):
    V2 = np.ascontiguousarray(
        np.asarray(V, dtype=np.float32).reshape(JB, F))
    Ln = np.ascontiguousarray(np.asarray(L, dtype=np.float32))
    qn = np.asarray(queue, dtype=np.float32).reshape(NCORES, QC, F)
    return [{"V2": V2, "L": Ln, "queue": np.ascontiguousarray(qn[i])}
            for i in range(NCORES)]


def _combine(outs):
    """outs: list of (2, 128, NT) arrays, one per core -> scalar loss."""
    pos_raw = outs[0][0].T.reshape(JB).astype(np.float64)   # jb = t*128 + p
    qsum = np.zeros(JB, dtype=np.float64)
    for o in outs:
        qsum += o[1].T.reshape(JB).astype(np.float64)
    pos_s = INV_T * pos_raw
    batch_sum = np.exp(pos_s).reshape(J, B).sum(axis=1)     # sum_i exp(pos[j,i])
    denom = np.log(np.repeat(batch_sum, B) + qsum)
    loss = -(pos_s.sum() - denom.sum()) / B
    return np.array(loss, dtype=np.float32)


def kernel(V, L, queue):
    res = _run(_make_in_maps(V, L, queue))
    return _combine([res.results[i]["out"] for i in range(NCORES)])


# revision 9
# speedup vs baseline: 1.0223x; 1.0223x over previous
"""Contrastive queue loss kernel for 8 Trainium2 NeuronCores.

Reference computation (all fp32):
    pos[j,b]    = V[j,b,:] . L[b,:] / T                  (J=2, B=256, F=128)
    qlog[j,b,q] = V[j,b,:] . queue[q,:] / T              (Q=65536)
    denom[j,b]  = log( sum_i exp(pos[j,i]) + sum_q exp(qlog[j,b,q]) )
    loss        = -sum_{j,b} (pos[j,b] - denom[j,b]) / B

Sharding: queue split along Q across 8 cores (8192 rows each); V replicated.
Each core emits its partial sum_q exp(10*logit[jb, q]).  pos (65K MACs) and
the final logsumexp combine run on the host in float64 — the device does the
33.5M-exp / 4.3-GFLOP queue part only.

Structural choices:
  * sum_q exp is invariant to q permutation -> queue shard is DMAed
    CONTIGUOUSLY (1-4KB per partition line), and the PE-transposed blocks
    come out q-permuted, which is fine.
  * V arrives pre-transposed from the host (V2T [f, jb]) so no on-device
    V transpose; only a tiny f32->bf16 cast.
  * The scalar engine's exp (1 elem/lane/cycle) is the roofline.  Part of
    the work (DVE_COLS of 8192 q-cols per jb tile) is offloaded to the
    Vector engine via the Schraudolph bit-trick exp:
        exp(10*x) ~= bitcast_f32(int32(x*EXP_A + EXP_B))
    with EXP_B pre-corrected so the mean multiplicative error over a
    uniform mantissa-fraction is 1.0 (validated: rel err ~3e-4 per jb
    row if used for ALL columns; we offload ~19%% so the loss-level
    error is ~1e-5, far under the 2e-2 gate).
  * Queue chunks ramp 128->1024 rows so the first ACT op issues ~3us
    after the tile body starts instead of waiting on a big first chunk.

Per-core dataflow:
  DMA queue chunk (contiguous fp32) -> DVE cast to bf16
  -> PE 128x128 identity-matmul transposes into PSUM -> DVE copy to SBUF
  -> PE matmul against persistent V2T (bf16) -> logits in PSUM (fp32)
  -> ACT exp(10x) with fused accumulation (accum_out), or DVE fast-exp
     (tensor_scalar mult/add -> int32, then reduce over the f32 bitcast)
  -> final column reduce, DMA out [128, NT].
"""

import numpy as np

J, B, F, Q = 2, 256, 128, 65536
NCORES = 8
QC = Q // NCORES          # 8192 queue rows per core
JB = J * B                # 512
INV_T = 10.0
NT = JB // 128            # 4 jb tiles of 128
TEMPERATURE = 0.1

# Schraudolph fast-exp constants: exp(10*x) = 2^(x*10*log2e) approximated by
# bitcast(int32(x*EXP_A + EXP_B)); EXP_B absorbs the +4.069% mean linear-
# interp bias (127*2^23 - log2(1.040690)*2^23).
EXP_A = float(np.float32(10.0 * 1.4426950408889634 * 8388608.0))
EXP_B = float(np.float32(127.0 * 8388608.0 - 0.0575361352 * 8388608.0))

# (row_start, nrows) queue chunks, contiguous per partition; small first
# chunks shorten the pipeline ramp to the first ACT op.
CHUNKS = [(0, 128), (128, 128), (256, 256), (512, 512),
          (1024, 512), (1536, 512)] + \
         [(r, 1024) for r in range(2048, QC, 1024)]
assert sum(nr for _, nr in CHUNKS) == QC

# Per-jb-tile consumer schedule over the 8192 q columns.  ('a', c0, c1):
# scalar-engine exp group; ('v', c0, c1): DVE fast-exp group.  Groups are
# <= 1536 cols (3 PSUM banks; pool bufs=2 + 2 staging banks = 8 banks).
SCHED = [
    ('a', 0, 128),
    ('a', 128, 1024),
    ('a', 1024, 2560),
    ('a', 2560, 4096),
    ('v', 4096, 5632),
    ('a', 5632, 7168),
    ('a', 7168, 8192),
]
assert all(c1 - c0 <= 1536 for _, c0, c1 in SCHED)
assert SCHED[0][1] == 0 and SCHED[-1][2] == QC
NG = len(SCHED)
N_WARMUP = 8              # dummy PE matmuls to lift the HAM clock gate

_STATE = {}


def _build():
    import concourse.tile as tile
    from concourse import bacc, masks, mybir

    f32 = mybir.dt.float32
    bf16 = mybir.dt.bfloat16
    i32 = mybir.dt.int32
    nc = bacc.Bacc("TRN2", target_bir_lowering=False, debug=False,
                   num_devices=None, enable_partition_id=False)

    vt_d = nc.dram_tensor("V2T", (128, JB), f32, kind="ExternalInput")
    q_d = nc.dram_tensor("queue", (QC, F), f32, kind="ExternalInput")
    # out[p, t] = sum over this core's queue shard of exp(10 * logit[jb, q]),
    # jb = t*128 + p
    out_d = nc.dram_tensor("out", (128, NT), f32, kind="ExternalOutput")

    # chunk column offsets (transposed q-cols land in chunk order)
    coff = []
    acc_cols = 0
    for _, nr in CHUNKS:
        coff.append(acc_cols)
        acc_cols += nr

    with tile.TileContext(nc) as tc:
        with (
            tc.tile_pool(name="const", bufs=1) as const_pool,
            tc.tile_pool(name="vl", bufs=1) as vl_pool,
            tc.tile_pool(name="qt", bufs=3) as qt_pool,
            tc.tile_pool(name="qtb", bufs=3) as qtb_pool,
            tc.tile_pool(name="qts", bufs=6) as qts_pool,
            tc.tile_pool(name="fex", bufs=2) as fex_pool,
            tc.tile_pool(name="res", bufs=1) as res_pool,
            tc.tile_pool(name="pslog", bufs=2, space="PSUM") as pslog_pool,
            tc.tile_pool(name="pst", bufs=2, space="PSUM") as pst_pool,
        ):
            # PE clock warmup while the first DMAs are in flight.
            wsrc = const_pool.tile([128, 128], bf16, tag="wsrc")
            nc.vector.memset(wsrc[:], 0.0)
            lgw = pslog_pool.tile([128, 128], f32, tag="pslog")
            for _ in range(N_WARMUP):
                nc.tensor.matmul(lgw[:], lhsT=wsrc[:], rhs=wsrc[:],
                                 start=True, stop=True)

            identb = const_pool.tile([128, 128], bf16, tag="identb")
            masks.make_identity(nc, identb[:])

            # V2T [f=128, jb=512] comes pre-transposed from the host.
            vt_f = vl_pool.tile([128, JB], f32)
            nc.sync.dma_start(vt_f[:], vt_d.ap())
            v2tb = vl_pool.tile([128, JB], bf16)
            nc.vector.tensor_copy(v2tb[:], vt_f[:])

            # ---- stream queue chunks: load (contiguous), cast, transpose ----
            segs = []                  # (qts_tile, tile_col0, global_col0, n)
            for ci, (r0, nr) in enumerate(CHUNKS):
                sr = nr // 128                         # rows per partition
                qt = qt_pool.tile([128, nr], f32, tag="qt")
                nc.sync.dma_start(
                    qt[:].rearrange("p (s f) -> p s f", f=F),
                    q_d.ap()[r0:r0 + nr, :].rearrange(
                        "(p s) f -> p s f", s=sr))
                qtb = qtb_pool.tile([128, nr], bf16, tag="qtb")
                nc.vector.tensor_copy(qtb[:], qt[:])
                pt = pst_pool.tile([128, nr], bf16, tag="pst")
                for s in range(nr // 128):
                    nc.tensor.transpose(
                        pt[:, s * 128:(s + 1) * 128],
                        qtb[:, s * 128:(s + 1) * 128], identb[:])
                qts = qts_pool.tile([128, nr], bf16, tag="qts")
                nc.vector.tensor_copy(qts[:], pt[:])
                segs.append((qts, coff[ci], nr))

            def emit_matmuls(lg, t, c0, c1):
                """Matmuls filling lg[:, 0:c1-c0] with logits for jb tile t,
                global q columns [c0, c1).  Each matmul is <= 512 wide (one
                PSUM bank)."""
                for qts, g0, n in segs:
                    o0, o1 = max(c0, g0), min(c1, g0 + n)
                    a = o0
                    while a < o1:
                        # stop at the next PSUM bank boundary of lg (512 f32)
                        b = min(o1, a + 512 - (a - c0) % 512)
                        nc.tensor.matmul(
                            lg[:, a - c0:b - c0],
                            lhsT=v2tb[:, t * 128:(t + 1) * 128],
                            rhs=qts[:, a - g0:b - g0], start=True, stop=True)
                        a = b

            # ---- logits + exp/accumulate (ACT) or fast-exp (DVE) ----
            # acc[p, t*NG + g] = partial sum for jb tile t, group g
            acc = res_pool.tile([128, NT * NG], f32)
            for gi, (eng, c0, c1) in enumerate(SCHED):
                w = c1 - c0
                for t in range(NT):
                    lg = pslog_pool.tile([128, w], f32, tag="pslog")
                    emit_matmuls(lg, t, c0, c1)
                    col = t * NG + gi
                    if eng == 'a':
                        nc.scalar.activation(
                            lg[:], lg[:], mybir.ActivationFunctionType.Exp,
                            scale=INV_T, accum_out=acc[:, col:col + 1])
                    else:
                        fx = fex_pool.tile([128, w], i32, tag="fex")
                        nc.vector.tensor_scalar(
                            fx[:], lg[:], EXP_A, EXP_B,
                            mybir.AluOpType.mult, mybir.AluOpType.add)
                        nc.vector.tensor_reduce(
                            out=acc[:, col:col + 1],
                            in_=fx[:].bitcast(f32),
                            axis=mybir.AxisListType.X,
                            op=mybir.AluOpType.add)

            # ---- finalize: reduce partials over groups, DMA out ----
            s_sb = res_pool.tile([128, NT], f32)
            for t in range(NT):
                nc.vector.tensor_reduce(
                    out=s_sb[:, t:t + 1],
                    in_=acc[:, t * NG:(t + 1) * NG],
                    axis=mybir.AxisListType.X, op=mybir.AluOpType.add)
            nc.sync.dma_start(out_d.ap(), s_sb[:])

    nc.compile()
    return nc


def _run(in_maps, trace=False, **kwargs):
    from concourse.bass_utils import run_bass_kernel_spmd
    if "nc" not in _STATE:
        _STATE["nc"] = _build()
    return run_bass_kernel_spmd(_STATE["nc"], in_maps, list(range(NCORES)),
                                trace=trace, **kwargs)


def _make_in_maps(V, L, queue):
    V2T = np.ascontiguousarray(
        np.asarray(V, dtype=np.float32).reshape(JB, F).T)
    qn = np.asarray(queue, dtype=np.float32).reshape(NCORES, QC, F)
    return [{"V2T": V2T, "queue": np.ascontiguousarray(qn[i])}
            for i in range(NCORES)]


def _combine(V, L, outs):
    """outs: list of (128, NT) qsum arrays, one per core -> scalar loss."""
    qsum = np.zeros(JB, dtype=np.float64)
    for o in outs:
        qsum += o.T.reshape(JB).astype(np.float64)   # jb = t*128 + p
    V2 = np.asarray(V, dtype=np.float64).reshape(JB, F)
    Ln = np.asarray(L, dtype=np.float64)
    pos = (V2.reshape(J, B, F) * Ln[None]).sum(-1).reshape(JB) / TEMPERATURE
    batch_sum = np.exp(pos).reshape(J, B).sum(axis=1)  # sum_i exp(pos[j,i])
    denom = np.log(np.repeat(batch_sum, B) + qsum)
    loss = -(pos.sum() - denom.sum()) / B
    return np.array(loss, dtype=np.float32)


def kernel(V, L, queue):
    res = _run(_make_in_maps(V, L, queue))
    return _combine(V, L, [res.results[i]["out"] for i in range(NCORES)])
